# revision 12
# baseline (speedup 1.0000x reference)
"""GNN message passing (2-layer, residual) on 8 TRN2 NeuronCores.

Strategy v4: shard destination nodes across 8 cores (12500 rows each).
Destination space is processed in superblocks of 4 blocks (512 rows) so
scatter slices are shared across 4 blocks: slices per superblock =
sum over 4 source windows of ceil(count/128) (~28), i.e. ~10% padding
instead of ~25% at block granularity. Neighbor rows (fp16, 256B) are
fetched with batched int16 `dma_gather` (one instruction per
(block-group, 25000-row source window)). Each slice is scattered with a
single 512-wide one-hot fp16 matmul into a full-bank PSUM tile
(aggT[feat, 512 dests]); per-block tails run the layer's linear/relu as
small fp16 matmuls. The serialized bottleneck is Q7 SWDGE descriptor
generation (~8.2ns/row) with DMA drain just below it; PE/DVE/ACT hide
underneath. Two launches; host concats h shards between them.
"""
import os
import sys
import types
import contextlib

import numpy as np

import concourse.bass as bass
import concourse.tile as tile
from concourse import bacc, mybir
from concourse.bass_utils import run_bass_kernel_spmd

N = 100000
E = 640000
D = 128
NC = 8
R = N // NC              # 12500 rows per core
NB = (R + 127) // 128    # 98 blocks; last block has 84 rows
NSB = (NB + 3) // 4      # 25 superblocks; last has 2 blocks
P = 128
NCH = 4                  # source windows (int16 idx < 32768)
CW = N // NCH            # 25000 rows per window
SBG = 6                  # superblocks per gather group
PAD_RL = 1023.0          # one-hot sentinel (exact in fp16)

PROFILE = bool(int(os.environ.get("GNN_PROFILE", "0")))
LAST_EXEC_NS = []


def _install_ntff_shim():
    if "antenv.axon_hooks" in sys.modules:
        return
    mod = types.ModuleType("antenv.axon_hooks")
    mod._hook = None
    mod.set_axon_ntff_profile_hook = lambda h: setattr(mod, "_hook", h)
    mod.get_axon_ntff_profile_hook = lambda: mod._hook
    sys.modules["antenv.axon_hooks"] = mod
    try:
        import antenv
        antenv.axon_hooks = mod
        from trn_agent_boot.trn_boot import _ntff_profile_via_ctypes
        mod.set_axon_ntff_profile_hook(
            _ntff_profile_via_ctypes("/opt/axon/libaxon_pjrt.so"))
    except Exception:
        pass


class Sched:
    pass


def _prep_edges(edge_index):
    """SPMD schedule: superblock/window-pure slices, shared by both layers.

    Column order is (group, window, superblock, t) so each (group, window)
    is one contiguous dma_gather. rl codes (dest-in-superblock, 0..511)
    are stored superblock-major so each slice's one-hot builds from one
    rl column.
    """
    row = np.asarray(edge_index[0], dtype=np.int64)
    col = np.asarray(edge_index[1], dtype=np.int64)
    k = row // R
    rloc = row - k * R
    b = rloc >> 7
    sb = b >> 2
    rl512 = (rloc & 511).astype(np.int32)     # dest within superblock
    c = col // CW
    ci = (col - c * CW).astype(np.int32)

    key = (k * NSB + sb) * NCH + c
    cnt = np.bincount(key, minlength=NC * NSB * NCH).reshape(NC, NSB, NCH)
    T = (cnt.max(axis=0) + P - 1) // P        # [NSB, NCH]
    T = np.maximum(T, 1)
    nbsb = T.sum(axis=1)                      # slices per superblock
    sbofs = np.zeros(NSB, dtype=np.int64)     # rl/slice col base per sb
    sbofs[1:] = np.cumsum(nbsb)[:-1]
    S = int(T.sum())

    # gather-column order: (group, window, sb, t)
    groups = [(g0, min(g0 + SBG, NSB)) for g0 in range(0, NSB, SBG)]
    gcol = np.zeros((NSB, NCH), dtype=np.int64)
    ginfo = []            # per group: (sb0, sb1, gstart, ncols, [(c, cstart, cn)])
    colp = 0
    for (sb0, sb1) in groups:
        gstart = colp
        gi = []
        for cc in range(NCH):
            start = colp
            for s in range(sb0, sb1):
                gcol[s, cc] = colp
                colp += T[s, cc]
            gi.append((cc, start, colp - start))
        ginfo.append((sb0, sb1, gstart, colp - gstart, gi))
    assert colp == S

    Tcum = np.zeros((NSB, NCH), dtype=np.int64)
    Tcum[:, 1:] = np.cumsum(T, axis=1)[:, :-1]

    flat_idx = np.zeros((NC, S * P), dtype=np.int16)
    rlT = np.full((NC, P, S), PAD_RL, dtype=np.float32)

    for kk in range(NC):
        m = k == kk
        sbk, ck, rlk, cik = sb[m], c[m], rl512[m], ci[m]
        order = np.lexsort((cik, ck, sbk))
        sbk, ck, rlk, cik = sbk[order], ck[order], rlk[order], cik[order]
        seg = sbk * NCH + ck
        cnts = np.bincount(seg, minlength=NSB * NCH)
        starts = np.zeros(NSB * NCH, dtype=np.int64)
        starts[1:] = np.cumsum(cnts)[:-1]
        pos = np.arange(len(sbk)) - starts[seg]
        t = pos >> 7
        p = pos & 127
        gc = gcol[sbk, ck] + t
        flat_idx[kk][gc * P + p] = cik
        rlc = sbofs[sbk] + Tcum[sbk, ck] + t
        rlT[kk][p, rlc] = rlk

    idxT = np.zeros((NC, P, S * 8), dtype=np.int16)
    for kk in range(NC):
        a16 = flat_idx[kk].reshape(-1, 16).T
        idxT[kk] = np.tile(a16, (8, 1))

    sc = Sched()
    sc.T, sc.nbsb, sc.sbofs, sc.S = T, nbsb, sbofs, S
    sc.gcol, sc.ginfo, sc.Tcum = gcol, ginfo, Tcum
    sc.idxT = idxT
    sc.rlT = rlT.astype(np.float16)
    return sc


def _common_prologue(nc, tc, ctx, sc, idx_d, rl_d):
    const = ctx.enter_context(tc.tile_pool(name="const", bufs=1))
    idxSB = const.tile([P, sc.S * 8], mybir.dt.int16)
    rlSB = const.tile([P, sc.S], mybir.dt.float16)
    # per-group idx loads so the first gather starts early
    for (sb0, sb1, gstart, ncols, gi) in sc.ginfo:
        nc.sync.dma_start(out=idxSB[:, gstart * 8:(gstart + ncols) * 8],
                          in_=idx_d[:, gstart * 8:(gstart + ncols) * 8])
    nc.sync.dma_start(out=rlSB[:], in_=rl_d[:])
    onesSB = const.tile([1, P], mybir.dt.float16)
    nc.vector.memset(onesSB[:], 1.0)
    iotaI = const.tile([P, 4, 512], mybir.dt.int32)
    nc.gpsimd.iota(iotaI[:], pattern=[[0, 4], [1, 512]], base=0,
                   channel_multiplier=0)
    iotaF = const.tile([P, 4, 512], mybir.dt.float16)
    nc.vector.tensor_copy(iotaF[:], iotaI[:])
    return const, idxSB, rlSB, onesSB, iotaF


def _emit_gathers(nc, gp, sc, src_d, idxSB):
    for (sb0, sb1, gstart, ncols, gi) in sc.ginfo:
        gt = gp.tile([P, ncols, D], mybir.dt.float16, tag="g")
        qn = 0
        for (cc, cstart, cn) in gi:
            if cn == 0:
                continue
            h0 = cn // 2
            for (cs, ch) in (((cstart, h0)) , ((cstart + h0, cn - h0))):
                if ch == 0:
                    continue
                n_idx = ch * P
                o0 = cs - gstart
                nc.gpsimd.dma_gather(
                    gt[:, o0:o0 + ch, :],
                    src_d[cc * CW:(cc + 1) * CW, :],
                    idxSB[:, cs * 8:(cs + ch) * 8],
                    n_idx, n_idx, D, single_packet=False,
                    queue_num=qn & 3)
                qn += 1
        yield sb0, sb1, gt, gstart


def _scatter_superblock(nc, sc, s, gt, gstart, mp, pa, rlSB, iotaF):
    """Accumulate aggT[feat, 512 dests] for superblock s in one PSUM bank.

    One-hot M tiles are built 4 slices per DVE op (fp16 2x mode); the
    superblock's rl columns are contiguous at sbofs[s].
    """
    psumA = pa.tile([P, 512], mybir.dt.float32, tag="pa")
    nslices = int(sc.nbsb[s])
    rl0 = int(sc.sbofs[s])
    # gather-col of each slice j (ordered by (window, t) == rl order)
    gcs = [int(sc.gcol[s, cc]) + t - gstart
           for cc in range(NCH) for t in range(int(sc.T[s, cc]))]
    for j0 in range(0, nslices, 4):
        nb = min(4, nslices - j0)
        M = mp.tile([P, 4, 512], mybir.dt.float16, tag="m")
        in0 = iotaF[:, :nb, :]
        in1 = rlSB[:, rl0 + j0:rl0 + j0 + nb].unsqueeze(2) \
            .broadcast_to((P, nb, 512))
        nc.vector.tensor_tensor(out=M[:, :nb, :], in0=in0, in1=in1,
                                op=mybir.AluOpType.is_equal)
        for jj in range(nb):
            j = j0 + jj
            nc.tensor.matmul(psumA[:], lhsT=gt[:, gcs[j], :],
                             rhs=M[:, jj, :],
                             start=(j == 0), stop=(j == nslices - 1))
    return psumA


def _build_layer0(sc):
    nc = bacc.Bacc("TRN2", target_bir_lowering=False, debug=False,
                   num_devices=NC, num_swdge_queues=4)
    x_d = nc.dram_tensor("x", [N, D], mybir.dt.float16, kind="ExternalInput")
    idx_d = nc.dram_tensor("idx", [P, sc.S * 8], mybir.dt.int16,
                           kind="ExternalInput")
    rl_d = nc.dram_tensor("rl", [P, sc.S], mybir.dt.float16,
                          kind="ExternalInput")
    w0_d = nc.dram_tensor("w0", [D, D], mybir.dt.float16, kind="ExternalInput")
    b0_d = nc.dram_tensor("b0", [1, D], mybir.dt.float16, kind="ExternalInput")
    h_d = nc.dram_tensor("h", [R, D], mybir.dt.float16, kind="ExternalOutput")

    with tile.TileContext(nc) as tc:
        with contextlib.ExitStack() as ctx:
            const, idxSB, rlSB, onesSB, iotaF = _common_prologue(
                nc, tc, ctx, sc, idx_d, rl_d)
            w0SB = const.tile([D, D], mybir.dt.float16)
            b0SB = const.tile([1, D], mybir.dt.float16)
            nc.sync.dma_start(out=w0SB[:], in_=w0_d[:])
            nc.sync.dma_start(out=b0SB[:], in_=b0_d[:])

            gp = ctx.enter_context(tc.tile_pool(name="gp", bufs=3))
            mp = ctx.enter_context(tc.tile_pool(name="mp", bufs=3))
            sp = ctx.enter_context(tc.tile_pool(name="sp", bufs=3))
            hp = ctx.enter_context(tc.tile_pool(name="hp", bufs=3))
            pa = ctx.enter_context(tc.tile_pool(name="pa", bufs=2, space="PSUM"))
            ph = ctx.enter_context(tc.tile_pool(name="ph", bufs=2, space="PSUM"))

            for sb0, sb1, gt, gstart in _emit_gathers(nc, gp, sc, x_d, idxSB):
                for s in range(sb0, sb1):
                    psumA = _scatter_superblock(nc, sc, s, gt, gstart, mp, pa,
                                                rlSB, iotaF)
                    for bb in range(4 * s, min(4 * s + 4, NB)):
                        bo = (bb & 3) * P
                        sA = sp.tile([P, P], mybir.dt.float16, tag="sa")
                        nc.scalar.activation(sA[:], psumA[:, bo:bo + P],
                                             mybir.ActivationFunctionType.Copy)
                        psumH = ph.tile([P, P], mybir.dt.float32, tag="phh")
                        nc.tensor.matmul(psumH[:], lhsT=sA[:], rhs=w0SB[:],
                                         start=True, stop=False)
                        nc.tensor.matmul(psumH[:], lhsT=onesSB[:], rhs=b0SB[:],
                                         start=False, stop=True)
                        hsb = hp.tile([P, P], mybir.dt.float16, tag="h")
                        nc.scalar.activation(hsb[:], psumH[:],
                                             mybir.ActivationFunctionType.Relu)
                        rows_b = min(P, R - bb * P)
                        nc.sync.dma_start(out=h_d[bb * P:bb * P + rows_b, :],
                                          in_=hsb[:rows_b, :])
    nc.compile()
    return nc


def _build_layer1(sc):
    nc = bacc.Bacc("TRN2", target_bir_lowering=False, debug=False,
                   num_devices=NC, num_swdge_queues=4)
    hf_d = nc.dram_tensor("hf", [N, D], mybir.dt.float16,
                          kind="ExternalInput")
    idx_d = nc.dram_tensor("idx", [P, sc.S * 8], mybir.dt.int16,
                           kind="ExternalInput")
    rl_d = nc.dram_tensor("rl", [P, sc.S], mybir.dt.float16,
                          kind="ExternalInput")
    w1_d = nc.dram_tensor("w1", [D, D], mybir.dt.float16, kind="ExternalInput")
    b1_d = nc.dram_tensor("b1", [P, 1], mybir.dt.float32, kind="ExternalInput")
    wp_d = nc.dram_tensor("wp", [D, D], mybir.dt.float16, kind="ExternalInput")
    bp_d = nc.dram_tensor("bp", [1, D], mybir.dt.float16, kind="ExternalInput")
    o_d = nc.dram_tensor("o", [R, D], mybir.dt.float32, kind="ExternalOutput")

    with tile.TileContext(nc) as tc:
        with contextlib.ExitStack() as ctx:
            const, idxSB, rlSB, onesSB, iotaF = _common_prologue(
                nc, tc, ctx, sc, idx_d, rl_d)
            w1SB = const.tile([D, D], mybir.dt.float16)
            b1SB = const.tile([P, 1], mybir.dt.float32)
            wpSB = const.tile([D, D], mybir.dt.float16)
            bpSB = const.tile([1, D], mybir.dt.float16)
            nc.sync.dma_start(out=w1SB[:], in_=w1_d[:])
            nc.sync.dma_start(out=b1SB[:], in_=b1_d[:])
            nc.sync.dma_start(out=wpSB[:], in_=wp_d[:])
            nc.sync.dma_start(out=bpSB[:], in_=bp_d[:])

            gp = ctx.enter_context(tc.tile_pool(name="gp", bufs=3))
            mp = ctx.enter_context(tc.tile_pool(name="mp", bufs=3))
            sp = ctx.enter_context(tc.tile_pool(name="sp", bufs=3))
            hp = ctx.enter_context(tc.tile_pool(name="hp", bufs=3))
            pa = ctx.enter_context(tc.tile_pool(name="pa", bufs=2, space="PSUM"))
            pz = ctx.enter_context(tc.tile_pool(name="pz", bufs=2, space="PSUM"))
            po = ctx.enter_context(tc.tile_pool(name="po", bufs=2, space="PSUM"))

            for sb0, sb1, gt, gstart in _emit_gathers(nc, gp, sc, hf_d, idxSB):
                for s in range(sb0, sb1):
                    psumA = _scatter_superblock(nc, sc, s, gt, gstart, mp, pa,
                                                rlSB, iotaF)
                    for bb in range(4 * s, min(4 * s + 4, NB)):
                        bo = (bb & 3) * P
                        sA1 = sp.tile([P, P], mybir.dt.float16, tag="sa")
                        nc.scalar.activation(sA1[:], psumA[:, bo:bo + P],
                                             mybir.ActivationFunctionType.Copy)
                        psumZ = pz.tile([P, P], mybir.dt.float32, tag="pz")
                        nc.tensor.matmul(psumZ[:], lhsT=w1SB[:], rhs=sA1[:],
                                         start=True, stop=True)
                        t1 = hp.tile([P, P], mybir.dt.float16, tag="t1")
                        nc.scalar.activation(t1[:], psumZ[:],
                                             mybir.ActivationFunctionType.Relu,
                                             bias=b1SB[:])
                        h2T = hp.tile([P, P], mybir.dt.float16, tag="h2")
                        nc.vector.tensor_add(h2T[:], t1[:], sA1[:])
                        psumO = po.tile([P, P], mybir.dt.float32, tag="po")
                        nc.tensor.matmul(psumO[:], lhsT=h2T[:], rhs=wpSB[:],
                                         start=True, stop=False)
                        nc.tensor.matmul(psumO[:], lhsT=onesSB[:], rhs=bpSB[:],
                                         start=False, stop=True)
                        osb = hp.tile([P, P], mybir.dt.float32, tag="o")
                        nc.scalar.activation(osb[:], psumO[:],
                                             mybir.ActivationFunctionType.Copy)
                        rows_b = min(P, R - bb * P)
                        nc.sync.dma_start(out=o_d[bb * P:bb * P + rows_b, :],
                                          in_=osb[:rows_b, :])
    nc.compile()
    return nc


def _run(nc, in_maps):
    global LAST_EXEC_NS
    res = run_bass_kernel_spmd(nc, in_maps, core_ids=list(range(NC)),
                               trace=PROFILE)
    if PROFILE:
        LAST_EXEC_NS.append(res.exec_time_ns)
    return res.results


def kernel(x, edge_index, W0, b0, W1, b1, Wp, bp):
    global LAST_EXEC_NS
    LAST_EXEC_NS = []
    if PROFILE:
        _install_ntff_shim()
    x16 = np.ascontiguousarray(np.asarray(x, dtype=np.float32)) \
        .astype(np.float16)
    W0h = np.asarray(W0, np.float32).astype(np.float16)
    W1h = np.asarray(W1, np.float32).astype(np.float16)
    Wph = np.asarray(Wp, np.float32).astype(np.float16)
    b0h = np.asarray(b0, np.float32).reshape(1, D).astype(np.float16)
    b1f = np.asarray(b1, np.float32).reshape(P, 1)
    bph = np.asarray(bp, np.float32).reshape(1, D).astype(np.float16)

    sc = _prep_edges(np.asarray(edge_index))

    nc0 = _build_layer0(sc)
    in0 = [{"x": x16, "idx": sc.idxT[k], "rl": sc.rlT[k],
            "w0": W0h, "b0": b0h} for k in range(NC)]
    res0 = _run(nc0, in0)
    hfull = np.concatenate([np.asarray(res0[k]["h"]) for k in range(NC)],
                           axis=0)

    nc1 = _build_layer1(sc)
    in1 = [{"hf": hfull, "idx": sc.idxT[k], "rl": sc.rlT[k],
            "w1": W1h, "b1": b1f, "wp": Wph, "bp": bph} for k in range(NC)]
    res1 = _run(nc1, in1)
    out = np.concatenate([np.asarray(res1[k]["o"], dtype=np.float32)
                          for k in range(NC)], axis=0)
    return out


# revision 14
# speedup vs baseline: 1.1422x; 1.1422x over previous
"""GNN message passing (2-layer, residual) on 8 TRN2 NeuronCores.

Strategy v4: shard destination nodes across 8 cores (12500 rows each).
Destination space is processed in superblocks of 4 blocks (512 rows) so
scatter slices are shared across 4 blocks: slices per superblock =
sum over 4 source windows of ceil(count/128) (~28), i.e. ~10% padding
instead of ~25% at block granularity. Neighbor rows (fp16, 256B) are
fetched with batched int16 `dma_gather` (one instruction per
(block-group, 25000-row source window)). Each slice is scattered with a
single 512-wide one-hot fp16 matmul into a full-bank PSUM tile
(aggT[feat, 512 dests]); per-block tails run the layer's linear/relu as
small fp16 matmuls. The serialized bottleneck is Q7 SWDGE descriptor
generation (~8.2ns/row) with DMA drain just below it; PE/DVE/ACT hide
underneath. Two launches; host concats h shards between them.
"""
import os
import sys
import types
import contextlib

import numpy as np

import concourse.bass as bass
import concourse.tile as tile
from concourse import bacc, mybir
from concourse.bass_utils import run_bass_kernel_spmd

N = 100000
E = 640000
D = 128
NC = 8
R = N // NC              # 12500 rows per core
NB = (R + 127) // 128    # 98 blocks; last block has 84 rows
NSB = (NB + 3) // 4      # 25 superblocks; last has 2 blocks
P = 128
NCH = 4                  # source windows (int16 idx < 32768)
CW = N // NCH            # 25000 rows per window
SBG = 6                  # superblocks per gather group
PAD_RL = 1023.0          # one-hot sentinel (exact in fp16)

PROFILE = bool(int(os.environ.get("GNN_PROFILE", "0")))
LAST_EXEC_NS = []


def _install_ntff_shim():
    if "antenv.axon_hooks" in sys.modules:
        return
    mod = types.ModuleType("antenv.axon_hooks")
    mod._hook = None
    mod.set_axon_ntff_profile_hook = lambda h: setattr(mod, "_hook", h)
    mod.get_axon_ntff_profile_hook = lambda: mod._hook
    sys.modules["antenv.axon_hooks"] = mod
    try:
        import antenv
        antenv.axon_hooks = mod
        from trn_agent_boot.trn_boot import _ntff_profile_via_ctypes
        mod.set_axon_ntff_profile_hook(
            _ntff_profile_via_ctypes("/opt/axon/libaxon_pjrt.so"))
    except Exception:
        pass


class Sched:
    pass


def _prep_edges(edge_index):
    """SPMD schedule: superblock/window-pure slices, shared by both layers.

    Column order is (group, window, superblock, t) so each (group, window)
    is one contiguous dma_gather. rl codes (dest-in-superblock, 0..511)
    are stored superblock-major so each slice's one-hot builds from one
    rl column.
    """
    row = np.asarray(edge_index[0], dtype=np.int64)
    col = np.asarray(edge_index[1], dtype=np.int64)
    k = row // R
    rloc = row - k * R
    b = rloc >> 7
    sb = b >> 2
    rl512 = (rloc & 511).astype(np.int32)     # dest within superblock
    c = col // CW
    ci = (col - c * CW).astype(np.int32)

    key = (k * NSB + sb) * NCH + c
    cnt = np.bincount(key, minlength=NC * NSB * NCH).reshape(NC, NSB, NCH)
    T = (cnt.max(axis=0) + P - 1) // P        # [NSB, NCH]
    T = np.maximum(T, 1)
    nbsb = T.sum(axis=1)                      # slices per superblock
    sbofs = np.zeros(NSB, dtype=np.int64)     # rl/slice col base per sb
    sbofs[1:] = np.cumsum(nbsb)[:-1]
    S = int(T.sum())

    # gather-column order: (group, window, sb, t)
    groups = [(g0, min(g0 + SBG, NSB)) for g0 in range(0, NSB, SBG)]
    gcol = np.zeros((NSB, NCH), dtype=np.int64)
    ginfo = []            # per group: (sb0, sb1, gstart, ncols, [(c, cstart, cn)])
    colp = 0
    for (sb0, sb1) in groups:
        gstart = colp
        gi = []
        for cc in range(NCH):
            start = colp
            for s in range(sb0, sb1):
                gcol[s, cc] = colp
                colp += T[s, cc]
            gi.append((cc, start, colp - start))
        ginfo.append((sb0, sb1, gstart, colp - gstart, gi))
    assert colp == S

    Tcum = np.zeros((NSB, NCH), dtype=np.int64)
    Tcum[:, 1:] = np.cumsum(T, axis=1)[:, :-1]

    flat_idx = np.zeros((NC, S * P), dtype=np.int16)
    rlT = np.full((NC, P, S), PAD_RL, dtype=np.float32)

    for kk in range(NC):
        m = k == kk
        sbk, ck, rlk, cik = sb[m], c[m], rl512[m], ci[m]
        order = np.lexsort((cik, ck, sbk))
        sbk, ck, rlk, cik = sbk[order], ck[order], rlk[order], cik[order]
        seg = sbk * NCH + ck
        cnts = np.bincount(seg, minlength=NSB * NCH)
        starts = np.zeros(NSB * NCH, dtype=np.int64)
        starts[1:] = np.cumsum(cnts)[:-1]
        pos = np.arange(len(sbk)) - starts[seg]
        t = pos >> 7
        p = pos & 127
        gc = gcol[sbk, ck] + t
        flat_idx[kk][gc * P + p] = cik
        rlc = sbofs[sbk] + Tcum[sbk, ck] + t
        rlT[kk][p, rlc] = rlk

    idxT = np.zeros((NC, P, S * 8), dtype=np.int16)
    for kk in range(NC):
        a16 = flat_idx[kk].reshape(-1, 16).T
        idxT[kk] = np.tile(a16, (8, 1))

    sc = Sched()
    sc.T, sc.nbsb, sc.sbofs, sc.S = T, nbsb, sbofs, S
    sc.gcol, sc.ginfo, sc.Tcum = gcol, ginfo, Tcum
    sc.idxT = idxT
    sc.rlT = rlT.astype(np.float16)
    return sc


def _common_prologue(nc, tc, ctx, sc, idx_d, rl_d):
    const = ctx.enter_context(tc.tile_pool(name="const", bufs=1))
    idxSB = const.tile([P, sc.S * 8], mybir.dt.int16)
    rlSB = const.tile([P, sc.S], mybir.dt.float16)
    # per-group idx loads so the first gather starts early
    for (sb0, sb1, gstart, ncols, gi) in sc.ginfo:
        nc.sync.dma_start(out=idxSB[:, gstart * 8:(gstart + ncols) * 8],
                          in_=idx_d[:, gstart * 8:(gstart + ncols) * 8])
    nc.sync.dma_start(out=rlSB[:], in_=rl_d[:])
    onesSB = const.tile([1, P], mybir.dt.float16)
    nc.vector.memset(onesSB[:], 1.0)
    iotaI = const.tile([P, 512], mybir.dt.int32)
    nc.gpsimd.iota(iotaI[:], pattern=[[1, 512]], base=0, channel_multiplier=0)
    iotaF = const.tile([P, 512], mybir.dt.float16)
    nc.vector.tensor_copy(iotaF[:], iotaI[:])
    return const, idxSB, rlSB, onesSB, iotaF


def _emit_gathers(nc, gp, sc, src_d, idxSB):
    for (sb0, sb1, gstart, ncols, gi) in sc.ginfo:
        gt = gp.tile([P, ncols, D], mybir.dt.float16, tag="g")
        for (cc, cstart, cn) in gi:
            if cn == 0:
                continue
            n_idx = cn * P
            o0 = cstart - gstart
            nc.gpsimd.dma_gather(
                gt[:, o0:o0 + cn, :],
                src_d[cc * CW:(cc + 1) * CW, :],
                idxSB[:, cstart * 8:(cstart + cn) * 8],
                n_idx, n_idx, D, single_packet=False,
                queue_num=cc)
        yield sb0, sb1, gt, gstart


def _scatter_superblock(nc, sc, s, gt, gstart, mp, pa, rlSB, iotaF):
    """Accumulate aggT[feat, 512 dests] for superblock s in one PSUM bank.

    One-hot M tiles are built 4 slices per DVE op (fp16 2x mode); the
    superblock's rl columns are contiguous at sbofs[s].
    """
    psumA = pa.tile([P, 512], mybir.dt.float32, tag="pa")
    nslices = int(sc.nbsb[s])
    rl0 = int(sc.sbofs[s])
    # gather-col of each slice j (ordered by (window, t) == rl order)
    gcs = [int(sc.gcol[s, cc]) + t - gstart
           for cc in range(NCH) for t in range(int(sc.T[s, cc]))]
    for j0 in range(0, nslices, 7):
        nb = min(7, nslices - j0)
        M = mp.tile([P, 7, 512], mybir.dt.float16, tag="m")
        in0 = iotaF[:].unsqueeze(1).broadcast_to((P, nb, 512))
        in1 = rlSB[:, rl0 + j0:rl0 + j0 + nb].unsqueeze(2) \
            .broadcast_to((P, nb, 512))
        nc.vector.tensor_tensor(out=M[:, :nb, :], in0=in0, in1=in1,
                                op=mybir.AluOpType.is_equal)
        for jj in range(nb):
            j = j0 + jj
            nc.tensor.matmul(psumA[:], lhsT=gt[:, gcs[j], :],
                             rhs=M[:, jj, :],
                             start=(j == 0), stop=(j == nslices - 1))
    return psumA


def _build_layer0(sc):
    nc = bacc.Bacc("TRN2", target_bir_lowering=False, debug=False,
                   num_devices=NC, num_swdge_queues=4)
    x_d = nc.dram_tensor("x", [N, D], mybir.dt.float16, kind="ExternalInput")
    idx_d = nc.dram_tensor("idx", [P, sc.S * 8], mybir.dt.int16,
                           kind="ExternalInput")
    rl_d = nc.dram_tensor("rl", [P, sc.S], mybir.dt.float16,
                          kind="ExternalInput")
    w0_d = nc.dram_tensor("w0", [D, D], mybir.dt.float16, kind="ExternalInput")
    b0_d = nc.dram_tensor("b0", [1, D], mybir.dt.float16, kind="ExternalInput")
    h_d = nc.dram_tensor("h", [R, D], mybir.dt.float16, kind="ExternalOutput")

    with tile.TileContext(nc) as tc:
        with contextlib.ExitStack() as ctx:
            const, idxSB, rlSB, onesSB, iotaF = _common_prologue(
                nc, tc, ctx, sc, idx_d, rl_d)
            w0SB = const.tile([D, D], mybir.dt.float16)
            b0SB = const.tile([1, D], mybir.dt.float16)
            nc.sync.dma_start(out=w0SB[:], in_=w0_d[:])
            nc.sync.dma_start(out=b0SB[:], in_=b0_d[:])

            gp = ctx.enter_context(tc.tile_pool(name="gp", bufs=3))
            mp = ctx.enter_context(tc.tile_pool(name="mp", bufs=3))
            sp = ctx.enter_context(tc.tile_pool(name="sp", bufs=3))
            hp = ctx.enter_context(tc.tile_pool(name="hp", bufs=3))
            pa = ctx.enter_context(tc.tile_pool(name="pa", bufs=2, space="PSUM"))
            ph = ctx.enter_context(tc.tile_pool(name="ph", bufs=2, space="PSUM"))

            for sb0, sb1, gt, gstart in _emit_gathers(nc, gp, sc, x_d, idxSB):
                for s in range(sb0, sb1):
                    psumA = _scatter_superblock(nc, sc, s, gt, gstart, mp, pa,
                                                rlSB, iotaF)
                    for bb in range(4 * s, min(4 * s + 4, NB)):
                        bo = (bb & 3) * P
                        sA = sp.tile([P, P], mybir.dt.float16, tag="sa")
                        nc.scalar.activation(sA[:], psumA[:, bo:bo + P],
                                             mybir.ActivationFunctionType.Copy)
                        psumH = ph.tile([P, P], mybir.dt.float32, tag="phh")
                        nc.tensor.matmul(psumH[:], lhsT=sA[:], rhs=w0SB[:],
                                         start=True, stop=False)
                        nc.tensor.matmul(psumH[:], lhsT=onesSB[:], rhs=b0SB[:],
                                         start=False, stop=True)
                        hsb = hp.tile([P, P], mybir.dt.float16, tag="h")
                        nc.scalar.activation(hsb[:], psumH[:],
                                             mybir.ActivationFunctionType.Relu)
                        rows_b = min(P, R - bb * P)
                        nc.sync.dma_start(out=h_d[bb * P:bb * P + rows_b, :],
                                          in_=hsb[:rows_b, :])
    nc.compile()
    return nc


def _build_layer1(sc):
    nc = bacc.Bacc("TRN2", target_bir_lowering=False, debug=False,
                   num_devices=NC, num_swdge_queues=4)
    hf_d = nc.dram_tensor("hf", [N, D], mybir.dt.float16,
                          kind="ExternalInput")
    idx_d = nc.dram_tensor("idx", [P, sc.S * 8], mybir.dt.int16,
                           kind="ExternalInput")
    rl_d = nc.dram_tensor("rl", [P, sc.S], mybir.dt.float16,
                          kind="ExternalInput")
    w1_d = nc.dram_tensor("w1", [D, D], mybir.dt.float16, kind="ExternalInput")
    b1_d = nc.dram_tensor("b1", [P, 1], mybir.dt.float32, kind="ExternalInput")
    wp_d = nc.dram_tensor("wp", [D, D], mybir.dt.float16, kind="ExternalInput")
    bp_d = nc.dram_tensor("bp", [1, D], mybir.dt.float16, kind="ExternalInput")
    o_d = nc.dram_tensor("o", [R, D], mybir.dt.float32, kind="ExternalOutput")

    with tile.TileContext(nc) as tc:
        with contextlib.ExitStack() as ctx:
            const, idxSB, rlSB, onesSB, iotaF = _common_prologue(
                nc, tc, ctx, sc, idx_d, rl_d)
            w1SB = const.tile([D, D], mybir.dt.float16)
            b1SB = const.tile([P, 1], mybir.dt.float32)
            wpSB = const.tile([D, D], mybir.dt.float16)
            bpSB = const.tile([1, D], mybir.dt.float16)
            nc.sync.dma_start(out=w1SB[:], in_=w1_d[:])
            nc.sync.dma_start(out=b1SB[:], in_=b1_d[:])
            nc.sync.dma_start(out=wpSB[:], in_=wp_d[:])
            nc.sync.dma_start(out=bpSB[:], in_=bp_d[:])

            gp = ctx.enter_context(tc.tile_pool(name="gp", bufs=3))
            mp = ctx.enter_context(tc.tile_pool(name="mp", bufs=3))
            sp = ctx.enter_context(tc.tile_pool(name="sp", bufs=3))
            hp = ctx.enter_context(tc.tile_pool(name="hp", bufs=3))
            pa = ctx.enter_context(tc.tile_pool(name="pa", bufs=2, space="PSUM"))
            pz = ctx.enter_context(tc.tile_pool(name="pz", bufs=2, space="PSUM"))
            po = ctx.enter_context(tc.tile_pool(name="po", bufs=2, space="PSUM"))

            for sb0, sb1, gt, gstart in _emit_gathers(nc, gp, sc, hf_d, idxSB):
                for s in range(sb0, sb1):
                    psumA = _scatter_superblock(nc, sc, s, gt, gstart, mp, pa,
                                                rlSB, iotaF)
                    for bb in range(4 * s, min(4 * s + 4, NB)):
                        bo = (bb & 3) * P
                        sA1 = sp.tile([P, P], mybir.dt.float16, tag="sa")
                        nc.scalar.activation(sA1[:], psumA[:, bo:bo + P],
                                             mybir.ActivationFunctionType.Copy)
                        psumZ = pz.tile([P, P], mybir.dt.float32, tag="pz")
                        nc.tensor.matmul(psumZ[:], lhsT=w1SB[:], rhs=sA1[:],
                                         start=True, stop=True)
                        t1 = hp.tile([P, P], mybir.dt.float16, tag="t1")
                        nc.scalar.activation(t1[:], psumZ[:],
                                             mybir.ActivationFunctionType.Relu,
                                             bias=b1SB[:])
                        h2T = hp.tile([P, P], mybir.dt.float16, tag="h2")
                        nc.vector.tensor_add(h2T[:], t1[:], sA1[:])
                        psumO = po.tile([P, P], mybir.dt.float32, tag="po")
                        nc.tensor.matmul(psumO[:], lhsT=h2T[:], rhs=wpSB[:],
                                         start=True, stop=False)
                        nc.tensor.matmul(psumO[:], lhsT=onesSB[:], rhs=bpSB[:],
                                         start=False, stop=True)
                        osb = hp.tile([P, P], mybir.dt.float32, tag="o")
                        nc.scalar.activation(osb[:], psumO[:],
                                             mybir.ActivationFunctionType.Copy)
                        rows_b = min(P, R - bb * P)
                        nc.sync.dma_start(out=o_d[bb * P:bb * P + rows_b, :],
                                          in_=osb[:rows_b, :])
    nc.compile()
    return nc


def _run(nc, in_maps):
    global LAST_EXEC_NS
    res = run_bass_kernel_spmd(nc, in_maps, core_ids=list(range(NC)),
                               trace=PROFILE)
    if PROFILE:
        LAST_EXEC_NS.append(res.exec_time_ns)
    return res.results


def kernel(x, edge_index, W0, b0, W1, b1, Wp, bp):
    global LAST_EXEC_NS
    LAST_EXEC_NS = []
    if PROFILE:
        _install_ntff_shim()
    x16 = np.ascontiguousarray(np.asarray(x, dtype=np.float32)) \
        .astype(np.float16)
    W0h = np.asarray(W0, np.float32).astype(np.float16)
    W1h = np.asarray(W1, np.float32).astype(np.float16)
    Wph = np.asarray(Wp, np.float32).astype(np.float16)
    b0h = np.asarray(b0, np.float32).reshape(1, D).astype(np.float16)
    b1f = np.asarray(b1, np.float32).reshape(P, 1)
    bph = np.asarray(bp, np.float32).reshape(1, D).astype(np.float16)

    sc = _prep_edges(np.asarray(edge_index))

    nc0 = _build_layer0(sc)
    in0 = [{"x": x16, "idx": sc.idxT[k], "rl": sc.rlT[k],
            "w0": W0h, "b0": b0h} for k in range(NC)]
    res0 = _run(nc0, in0)
    hfull = np.concatenate([np.asarray(res0[k]["h"]) for k in range(NC)],
                           axis=0)

    nc1 = _build_layer1(sc)
    in1 = [{"hf": hfull, "idx": sc.idxT[k], "rl": sc.rlT[k],
            "w1": W1h, "b1": b1f, "wp": Wph, "bp": bph} for k in range(NC)]
    res1 = _run(nc1, in1)
    out = np.concatenate([np.asarray(res1[k]["o"], dtype=np.float32)
                          for k in range(NC)], axis=0)
    return out
